# revision 24
# baseline (speedup 1.0000x reference)
"""Trainium2 Bass kernel for nn_CdfgReader (GNN message passing).

Strategy: the B=64 samples reference only G=8 distinct graphs, and the whole
GNN stack (input dense + 4 message-passing layers + softmax + residual) depends
only on the graph, not the sample. So each of the 8 NeuronCores computes the
full GNN for ONE graph g in [N=1024, H=256]. The per-sample masked mean is a
final [N,B]x[N,H] matmul against a host-built 0/1 mask matrix (rows zeroed for
samples of other graphs); the host sums the row-disjoint [B,H] partial
outputs and divides by the per-sample node count.

Matmul layouts avoid any on-device transpose:
  - layer: t = (A @ x)^T = matmul(lhsT=x, rhs=A^T)   (A^T fed from host)
  -        h = t^T @ W    = matmul(lhsT=t, rhs=W)
  - input: x0 = xs @ W_in = matmul(lhsT=xs^T, rhs=W_in)
  - out:   o = matmul(lhsT=maskT, rhs=x_final)

Fast path (biases zero, as in this problem): A is rescaled x20 on the host so
its entries become exactly-representable 0/1 fp8; both the A-matmul and the
W-matmul run fp8 DoubleRow; the x20 is undone by activation scale=0.05 on the
PSUM read. The mean path is also fp8; the input dense runs bf16 (fp8 xs/W_in
measurably hurts accuracy).

Schedule notes (from ntff trace analysis of the 47.8us baseline):
  - every HBM->SBUF load's completion semaphore reaches its final tick ~2.2us
    after the data lands (stores don't lag), so the DMA plan spreads the 7
    loads across the three DMA-capable queues (sync/scalar/gpsimd) ordered
    by first-consumption time: xw halves lead on sync, aT j0-3 on scalar,
    aT j4-7 on gpsimd, mT+ws trail on sync.
  - the PE clock needs ~3us of CONTINUOUS busy to reach 2.4GHz (0.65/1.2GHz
    pstates below that), and any PE idle resets the ramp; 13 dummy matmuls
    run back-to-back from the preamble until the input dense's DMA semaphore
    releases (~13us, jittery), so the ramp happens once, early.
  - W_in is stored FIRST in xw so the input dense's rhs is in the earliest
    bytes; xw is split so the first chunk releases the first 4 dense tiles.
  - engine op cost is ~200ns fixed + ~1ns/elem (vector) while scalar
    ACTIVATE scales sub-linearly; casts split vector/scalar accordingly.
  - the last layer reserves scalar for the softmax exp chain (the endgame
    bottleneck): its nch=1 chain casts go to vector, the W matmuls + exps
    carry scheduler priority, and the masked softmax mean accumulates in
    two PSUM banks so the first store overlaps the second exp half.
  - masked-mean matmuls use DoubleRow (contract 2 node tiles per pass).
  - softmax's 1/sum (~1/256, subnormal in fp8) is scaled x64 into the mask
    tile and divided back out on the host; softmax skips the max-subtraction
    (|logits| < 1 by construction).
  - NOTE for future tuning: measured exec time includes ~6.5us fixed engine
    preamble and ~8us semaphore-reset teardown, and the device thermally
    drifts +1-3us when runs are <2min apart — A/B comparisons need
    interleaved cold runs.
"""

import numpy as np
import ml_dtypes

from concourse import bacc
import concourse.mybir as mybir
import concourse.tile as tile
from concourse.bass_utils import run_bass_kernel_spmd

G, N, F, H, L, B = 8, 1024, 128, 256, 4, 64
P = 128
NT = N // P   # 8 node tiles
HT = H // P   # 2 hidden tiles
NCH = N // 512  # 2 free-dim chunks of 512 for the big matmul
NCORES = 8

F32 = mybir.dt.float32
BF16 = mybir.dt.bfloat16
F8 = mybir.dt.float8e4
PM_DR = mybir.MatmulPerfMode.DoubleRow
AX = mybir.AxisListType.X
AF = mybir.ActivationFunctionType
MUL = mybir.AluOpType.mult
MAX = mybir.AluOpType.max

# softmax 1/sum is ~1/256 — subnormal in fp8e4m3 — so the device computes
# mask*(1/sum)*MTS_SCALE and the host divides the late partials by MTS_SCALE
MTS_SCALE = 64.0

NDUM = 8  # PE-warmup matmuls: keep the clock ramping until the dense starts

_NCS = {}


def _build_nc_fast():
    """Biasless fast path: fp8 DoubleRow A- and W-matmuls, fp8 mean path."""
    nc = bacc.Bacc()
    # xw = [W_in | xs^T]: W_in leads so the dense rhs is in the first chunk
    xw = nc.dram_tensor("xw", [F, H + N], BF16, kind="ExternalInput")
    # host-pretiled aT: aT[p, j*N+n] = (A^T * 20)[j*P+p, n], fp8 0/1
    aT = nc.dram_tensor("aT", [P, NT * N], F8, kind="ExternalInput")
    # host-pretiled Ws (unscaled): ws[p, ((l*HT+c)*H)+h] = Ws[l, c*P+p, h]
    ws = nc.dram_tensor("ws", [P, L * HT * H], F8, kind="ExternalInput")
    # host-pretiled 0/1 mask (unscaled): mT[p, j*B+b] = mask[b, j*P+p]
    mT = nc.dram_tensor("mT", [P, NT * B], F8, kind="ExternalInput")
    outa = nc.dram_tensor("outa", [B, H], F32, kind="ExternalOutput")
    outb0 = nc.dram_tensor("outb0", [B, H], BF16, kind="ExternalOutput")
    outb1 = nc.dram_tensor("outb1", [B, H], BF16, kind="ExternalOutput")

    with tile.TileContext(nc) as tc:
        with (
            tc.tile_pool(name="const", bufs=1) as const,
            tc.tile_pool(name="state", bufs=2) as state,
            tc.tile_pool(name="scratch", bufs=3) as scratch,
            tc.tile_pool(name="epool", bufs=8) as epool,
            tc.tile_pool(name="mpool", bufs=8) as mpool,
            tc.tile_pool(name="ps_t", bufs=4, space="PSUM") as ps_t,
            tc.tile_pool(name="ps_h", bufs=4, space="PSUM") as ps_h,
        ):
            # ---- DMA loads: one per queue, ordered by first consumption.
            # sync: xw (split so the first dense tiles release early), mT, ws
            # scalar: aT j0-1, j2-3   gpsimd: aT j4-5, j6-7 ----
            # one big load per queue: the FIRST transfer on each queue
            # completes with fast semaphore ticks (~data rate), while later
            # transfers tick ~4x slower under notification contention — so
            # the three large loads each take a queue's fast slot
            xw_sb = const.tile([P, H + N], BF16)
            nc.sync.dma_start(xw_sb[:], xw[:])
            win_sb = xw_sb[:, 0:H]
            at_sb = const.tile([P, NT, N], F8)
            mt_sb = const.tile([P, NT, B], F8)
            ws_sb = const.tile([P, L * HT, H], F8)
            atr = aT.rearrange("p (o n) -> p o n", n=N)
            nc.scalar.dma_start(at_sb[:, 0:4, :], atr[:, 0:4, :])
            nc.gpsimd.dma_start(at_sb[:, 4:8, :], atr[:, 4:8, :])
            nc.sync.dma_start(mt_sb[:], mT.rearrange("p (o b) -> p o b", b=B))
            nc.sync.dma_start(ws_sb[:], ws.rearrange("p (c h) -> p c h", h=H))

            # ---- Exp activation-table preload (after scalar's DMA issue) ----
            warm = scratch.tile([P, 1], F32, tag="warm")
            nc.vector.memset(warm[:], 0.0)
            warm2 = scratch.tile([P, 1], F32, tag="warm2")
            nc.scalar.activation(warm2[:], warm[:], AF.Exp)

            # ---- PE warm-up: DMA-independent dummy matmuls keep the PE busy
            # so the clock ramp (0.65->1.2->2.4GHz over ~3us of continuous
            # execution) completes before the input dense ----
            dum_w = scratch.tile([P, 64], BF16, tag="dumw")
            nc.vector.memset(dum_w[:], 0.0)
            dum_r = scratch.tile([P, 512], BF16, tag="dumr")
            nc.vector.memset(dum_r[:], 0.0)
            for _ in range(NDUM):
                pdum = ps_t.tile([64, 512], F32, tag="ps_t")
                nc.tensor.matmul(pdum[:], dum_w[:], dum_r[:],
                                 start=True, stop=True)

            # ---- input dense: x0 = relu(xs @ W_in), bf16 in fp8 out ----
            x0b_sb = const.tile([P, NT, H], F8)
            for p in range(NT):
                ps = ps_h.tile([P, H], F32, tag="ps_h")
                nc.tensor.matmul(
                    ps[:], xw_sb[:, H + p * P:H + (p + 1) * P], win_sb,
                    start=True, stop=True,
                )
                # alternate relu engines so x0 j-pairs resolve fast
                if p % 2 == 0:
                    nc.vector.tensor_scalar_max(x0b_sb[:, p, :], ps[:], 0.0)
                else:
                    nc.scalar.activation(x0b_sb[:, p, :], ps[:], AF.Relu)

            x_cur = x0b_sb  # fp8 [P, NT, H]

            def cast_chain(i, nch, t_sb, ps, last=False):
                # split each PSUM->SBUF cast across vector+scalar in parallel;
                # in the last layer scalar is reserved for the softmax exps,
                # so both pieces go to vector
                base = nch * 512
                if last and nch == 1:
                    # scalar is mid-exp-chain by now; keep it free
                    nc.vector.tensor_copy(
                        t_sb[:, i, base:base + 256], ps[:, 0:256]
                    )
                    nc.vector.tensor_copy(
                        t_sb[:, i, base + 256:base + 512], ps[:, 256:512]
                    )
                elif i == 0:
                    nc.vector.tensor_copy(
                        t_sb[:, i, base:base + 256], ps[:, 0:256]
                    )
                    nc.scalar.activation(
                        t_sb[:, i, base + 256:base + 512], ps[:, 256:512],
                        AF.Copy
                    )
                else:
                    # swap engines for the i=1 chain: the W matmuls for a
                    # p-half need one piece from EACH chain, so alternating
                    # engines lets both pieces cast in parallel
                    nc.scalar.activation(
                        t_sb[:, i, base:base + 256], ps[:, 0:256], AF.Copy
                    )
                    nc.vector.tensor_copy(
                        t_sb[:, i, base + 256:base + 512], ps[:, 256:512]
                    )

            def w_relu(p, x_new, ps):
                if p % 2 == 0:
                    nc.scalar.activation(
                        x_new[:, p, :], ps[:], AF.Relu, scale=1.0 / 20.0
                    )
                else:
                    nc.vector.tensor_scalar(
                        x_new[:, p, :], ps[:], 1.0 / 20.0, 0.0, MUL, MAX
                    )

            def w_matmul(l, p):
                # all W matmuls cycle ps_h (drained by relus/exps mid-phase);
                # keeping ps_t chain-only means the next layer's first A-chain
                # reuses an early-cast-freed bank instead of WAR-waiting on
                # W p4's relu
                ps = ps_h.tile([P, H], F32, tag="ps_h")
                nc.tensor.matmul(
                    ps[:],
                    t_sb[:, 0:2, p * P:(p + 1) * P],
                    ws_sb[:, l * HT:l * HT + 2, :],
                    start=True, stop=True, perf_mode=PM_DR,
                )
                return ps

            # last-layer softmax pieces
            es, mts = [], []

            def softmax_exp(l, p):
                ps = w_matmul(l, p)
                e = epool.tile([P, H], F8, tag="e")
                ssum = scratch.tile([P, 1], F32, tag="ssum")
                nc.scalar.activation(
                    e[:], ps[:], AF.Exp, scale=1.0 / 20.0, accum_out=ssum[:],
                )
                es.append(e)
                return ssum

            def softmax_norm(p, ssum):
                rinv = scratch.tile([P, 1], F32, tag="rinv")
                nc.vector.reciprocal(rinv[:], ssum[:])
                mt = mpool.tile([P, B], F8, tag="mts")
                nc.vector.tensor_scalar(
                    mt[:], mt_sb[:, p, :], rinv[:], MTS_SCALE, MUL, MUL,
                )
                mts.append(mt)

            # ---- message-passing layers ----
            for l in range(L):
                t_sb = state.tile([P, HT, N], F8, tag="t")
                if l == 0:
                    # j-outer: consume at tiles as the DMA delivers them
                    CH = [(0, 0), (1, 0), (0, 1), (1, 1)]
                    chains = {}
                    for i, nch in CH:
                        chains[(i, nch)] = ps_t.tile(
                            [P, 512], F32, tag="ps_t", name=f"pt0_{i}{nch}"
                        )
                    for j in range(0, NT, 2):
                        for i, nch in CH:
                            nc.tensor.matmul(
                                chains[(i, nch)][:],
                                x_cur[:, j:j + 2, i * P:(i + 1) * P].opt(),
                                at_sb[:, j:j + 2, nch * 512:(nch + 1) * 512].opt(),
                                start=(j == 0), stop=(j + 2 == NT),
                                perf_mode=PM_DR,
                            )
                    # masked mean, part 1: pso_a = mT^T @ x0 (DoubleRow)
                    # fills the cast gap on the PE; its store overlaps the
                    # remaining layers
                    pso_a = ps_h.tile([B, H], F32, tag="ps_h")
                    for j in range(0, NT, 2):
                        nc.tensor.matmul(
                            pso_a[:],
                            mt_sb[:, j:j + 2, :],
                            x_cur[:, j:j + 2, :],
                            start=(j == 0), stop=(j + 2 == NT),
                            perf_mode=PM_DR,
                        )
                    for i, nch in CH:
                        cast_chain(i, nch, t_sb, chains[(i, nch)])
                    oa_sb = scratch.tile([B, H], F32, tag="oa")
                    nc.vector.tensor_copy(oa_sb[:], pso_a[:])
                    nc.sync.dma_start(outa[:], oa_sb[:])
                    x_new = state.tile([P, NT, H], F8, tag="x")
                    for p in range(NT):
                        w_relu(p, x_new, w_matmul(l, p))
                    x_cur = x_new
                else:
                    # half-interleaved: A-half nch, casts, W p-half, repeat.
                    # The W matmuls' cast deps resolve while the second
                    # A-half streams, so the PE never drains.
                    last = l == L - 1
                    x_new = None
                    if not last:
                        x_new = state.tile([P, NT, H], F8, tag="x")
                    for nch in range(NCH):
                        for i in range(HT):
                            ps = ps_t.tile([P, 512], F32, tag="ps_t")
                            for j in range(0, NT, 2):
                                nc.tensor.matmul(
                                    ps[:],
                                    x_cur[:, j:j + 2, i * P:(i + 1) * P].opt(),
                                    at_sb[:, j:j + 2, nch * 512:(nch + 1) * 512].opt(),
                                    start=(j == 0), stop=(j + 2 == NT),
                                    perf_mode=PM_DR,
                                )
                            cast_chain(i, nch, t_sb, ps, last=last)
                        # (a) the last layer's W matmuls + exps get scheduler
                        # priority so they slot into the A-stream as soon as
                        # their cast deps resolve: the scalar exp chain (the
                        # endgame bottleneck) then starts mid-A-phase
                        if last:
                            ssums = []
                            with tc.high_priority():
                                for p in range(nch * 4, nch * 4 + 4):
                                    ssums.append(softmax_exp(l, p))
                            for k, p in enumerate(range(nch * 4, nch * 4 + 4)):
                                softmax_norm(p, ssums[k])
                        else:
                            for p in range(nch * 4, nch * 4 + 4):
                                w_relu(p, x_new, w_matmul(l, p))
                        if last and nch == 0:
                            # masked softmax mean, first half: accumulate
                            # p0-3 while the second A-half runs, store early
                            pso_b0 = ps_t.tile([B, H], F32, tag="ps_t")
                            for p in range(4):
                                nc.tensor.matmul(
                                    pso_b0[:], mts[p][:], es[p][:],
                                    start=(p == 0), stop=(p == 3),
                                )
                            ob0_sb = scratch.tile([B, H], BF16, tag="ob0")
                            nc.vector.tensor_copy(ob0_sb[:], pso_b0[:])
                            nc.gpsimd.dma_start(outb0[:], ob0_sb[:])
                    if not last:
                        x_cur = x_new

            # masked softmax mean, second half
            pso_b1 = ps_t.tile([B, H], F32, tag="ps_t")
            for p in range(4, NT):
                nc.tensor.matmul(
                    pso_b1[:], mts[p][:], es[p][:],
                    start=(p == 4), stop=(p == NT - 1),
                )
            ob1_sb = scratch.tile([B, H], BF16, tag="ob1")
            nc.vector.tensor_copy(ob1_sb[:], pso_b1[:])
            nc.sync.dma_start(outb1[:], ob1_sb[:])

    nc.compile()
    return nc


def _build_nc_biased():
    """General path (nonzero biases): all-f32r, bias adds on DVE."""
    F32R = mybir.dt.float32r
    nc = bacc.Bacc()
    xT = nc.dram_tensor("xT", [F, N], F32R, kind="ExternalInput")
    aT = nc.dram_tensor("aT", [N, N], F32R, kind="ExternalInput")
    win = nc.dram_tensor("win", [F, H], F32R, kind="ExternalInput")
    bin_ = nc.dram_tensor("bin", [H], F32, kind="ExternalInput")
    ws = nc.dram_tensor("ws", [L, H, H], F32R, kind="ExternalInput")
    bsd = nc.dram_tensor("bs", [L, H], F32, kind="ExternalInput")
    mT = nc.dram_tensor("mT", [N, B], F32R, kind="ExternalInput")
    out = nc.dram_tensor("out", [B, H], F32, kind="ExternalOutput")

    with tile.TileContext(nc) as tc:
        with (
            tc.tile_pool(name="const", bufs=1) as const,
            tc.tile_pool(name="state", bufs=2) as state,
            tc.tile_pool(name="scratch", bufs=3) as scratch,
            tc.tile_pool(name="ps_t", bufs=4, space="PSUM") as ps_t,
            tc.tile_pool(name="ps_h", bufs=4, space="PSUM") as ps_h,
        ):
            xt_sb = const.tile([P, N], F32R)
            nc.sync.dma_start(xt_sb[:], xT[:])
            win_sb = const.tile([P, H], F32R)
            nc.sync.dma_start(win_sb[:], win[:])
            mt_sb = const.tile([P, NT, B], F32R)
            nc.sync.dma_start(mt_sb[:], mT.rearrange("(o p) b -> p o b", p=P))
            ws_sb = const.tile([P, L * HT, H], F32R)
            nc.sync.dma_start(ws_sb[:], ws.rearrange("l (c p) h -> p (l c) h", p=P))
            bin_sb = const.tile([P, H], F32)
            nc.sync.dma_start(bin_sb[:], bin_[None, :].broadcast_to([P, H]))
            bs_sb = const.tile([P, L, H], F32)
            for l in range(L):
                nc.sync.dma_start(
                    bs_sb[:, l, :], bsd[l][None, :].broadcast_to([P, H])
                )
            at_sb = const.tile([P, NT, N], F32R)
            for j in range(NT):
                nc.sync.dma_start(at_sb[:, j, :], aT[j * P:(j + 1) * P, :])

            x0_sb = const.tile([P, NT, H], F32R)
            for p in range(NT):
                ps = ps_h.tile([P, H], F32, tag="ps_h")
                nc.tensor.matmul(
                    ps[:], xt_sb[:, p * P:(p + 1) * P], win_sb[:],
                    start=True, stop=True,
                )
                h = scratch.tile([P, H], F32, tag="hadd")
                nc.vector.tensor_add(h[:], ps[:], bin_sb[:])
                nc.scalar.activation(x0_sb[:, p, :], h[:], AF.Relu)

            x_cur = x0_sb

            for l in range(L):
                t_sb = state.tile([P, HT, N], F32R, tag="t")
                for i in range(HT):
                    for nch in range(NCH):
                        ps = ps_t.tile([P, 512], F32, tag="ps_t")
                        for j in range(NT):
                            nc.tensor.matmul(
                                ps[:],
                                x_cur[:, j, i * P:(i + 1) * P],
                                at_sb[:, j, nch * 512:(nch + 1) * 512],
                                start=(j == 0), stop=(j == NT - 1),
                            )
                        nc.any.tensor_copy(
                            t_sb[:, i, nch * 512:(nch + 1) * 512], ps[:]
                        )
                x_new = state.tile([P, NT, H], F32R, tag="x")
                for p in range(NT):
                    ps = ps_h.tile([P, H], F32, tag="ps_h")
                    for c in range(HT):
                        nc.tensor.matmul(
                            ps[:],
                            t_sb[:, c, p * P:(p + 1) * P],
                            ws_sb[:, l * HT + c, :],
                            start=(c == 0), stop=(c == HT - 1),
                        )
                    h = scratch.tile([P, H], F32, tag="hadd")
                    nc.vector.tensor_add(h[:], ps[:], bs_sb[:, l, :])
                    if l < L - 1:
                        nc.scalar.activation(x_new[:, p, :], h[:], AF.Relu)
                    else:
                        negmax = scratch.tile([P, 1], F32, tag="negmax")
                        nc.vector.reduce_max(negmax[:], h[:], axis=AX, negate=True)
                        e = scratch.tile([P, H], F32, tag="e")
                        ssum = scratch.tile([P, 1], F32, tag="ssum")
                        nc.scalar.activation(
                            e[:], h[:], AF.Exp, bias=negmax[:], accum_out=ssum[:]
                        )
                        rinv = scratch.tile([P, 1], F32, tag="rinv")
                        nc.vector.reciprocal(rinv[:], ssum[:])
                        sm = scratch.tile([P, H], F32, tag="sm")
                        nc.vector.tensor_scalar_mul(sm[:], e[:], rinv[:])
                        nc.vector.tensor_add(x_new[:, p, :], sm[:], x0_sb[:, p, :])
                x_cur = x_new

            pso = ps_h.tile([B, H], F32, tag="ps_h")
            for j in range(NT):
                nc.tensor.matmul(
                    pso[:], mt_sb[:, j, :], x_cur[:, j, :],
                    start=(j == 0), stop=(j == NT - 1),
                )
            o_sb = scratch.tile([B, H], F32, tag="o")
            nc.any.tensor_copy(o_sb[:], pso[:])
            nc.sync.dma_start(out[:], o_sb[:])

    nc.compile()
    return nc


def get_nc(variant):
    if variant not in _NCS:
        if variant == "fast8":
            _NCS[variant] = _build_nc_fast()
        else:
            _NCS[variant] = _build_nc_biased()
    return _NCS[variant]


def make_in_maps(graph, coverpoint_mask, cdfg_xs, cdfg_as, W_in, b_in, Ws, bs,
                 variant):
    graph = np.asarray(graph)
    mask = np.asarray(coverpoint_mask)
    xs = np.ascontiguousarray(np.asarray(cdfg_xs, dtype=np.float32))
    As = np.asarray(cdfg_as, dtype=np.float32)
    W_in = np.ascontiguousarray(np.asarray(W_in, dtype=np.float32))
    b_in = np.ascontiguousarray(np.asarray(b_in, dtype=np.float32))
    Ws = np.ascontiguousarray(np.asarray(Ws, dtype=np.float32))
    bs = np.ascontiguousarray(np.asarray(bs, dtype=np.float32))

    if variant == "fast8":
        # [P, L*HT*H]: ws_t[p, ((l*HT+c)*H)+h] = Ws[l, c*P+p, h]  (unscaled)
        ws_dev = np.ascontiguousarray(
            Ws.reshape(L, HT, P, H)
            .transpose(2, 0, 1, 3)
            .reshape(P, L * HT * H)
            .astype(ml_dtypes.float8_e4m3)
        )
        win_dev = W_in.astype(ml_dtypes.bfloat16)
    else:
        cnt = np.maximum(mask.sum(axis=1), 1.0).astype(np.float32)
        scaled = mask.astype(np.float32) / cnt[:, None]

    in_maps = []
    for g in range(NCORES):
        sel = graph == g
        if variant == "fast8":
            mTg = np.where(sel[:, None], mask, False).T.astype(np.float32)
            m = {
                "xw": np.ascontiguousarray(
                    np.concatenate(
                        [win_dev, xs[g].T.astype(ml_dtypes.bfloat16)],
                        axis=1,
                    )
                ),
                "ws": ws_dev,
                # [P, NT*N]: aT_t[p, j*N+n] = (A^T*20)[j*P+p, n], exact 0/1 fp8
                "aT": np.ascontiguousarray(
                    (As[g].T * 20.0)
                    .reshape(NT, P, N)
                    .transpose(1, 0, 2)
                    .reshape(P, NT * N)
                    .astype(ml_dtypes.float8_e4m3)
                ),
                # [P, NT*B]: mt_t[p, j*B+b] = mTg[j*P+p, b], exact 0/1 fp8
                "mT": np.ascontiguousarray(
                    mTg.reshape(NT, P, B)
                    .transpose(1, 0, 2)
                    .reshape(P, NT * B)
                    .astype(ml_dtypes.float8_e4m3)
                ),
            }
        else:
            mTg = np.ascontiguousarray(np.where(sel[:, None], scaled, 0.0).T)
            m = {
                "xT": np.ascontiguousarray(xs[g].T),
                "win": W_in,
                "mT": mTg.astype(np.float32),
                "aT": np.ascontiguousarray(As[g].T),
                "ws": Ws,
                "bin": b_in,
                "bs": bs,
            }
        in_maps.append(m)
    return in_maps


def kernel(graph, coverpoint_mask, cdfg_xs, cdfg_as, W_in, b_in, Ws, bs,
           **run_kwargs):
    biasless = not (np.any(np.asarray(b_in)) or np.any(np.asarray(bs)))
    variant = "fast8" if biasless else "biased"
    in_maps = make_in_maps(
        graph, coverpoint_mask, cdfg_xs, cdfg_as, W_in, b_in, Ws, bs, variant
    )
    nc = get_nc(variant)
    res = run_bass_kernel_spmd(
        nc, in_maps, core_ids=list(range(NCORES)), **run_kwargs
    )
    if variant == "fast8":
        out = np.zeros((B, H), dtype=np.float32)
        for r in res.results:
            out += r["outa"]
            out += (r["outb0"].astype(np.float32)
                    + r["outb1"].astype(np.float32)) / MTS_SCALE
        cnt = np.maximum(
            np.asarray(coverpoint_mask).sum(axis=1), 1.0
        ).astype(np.float32)
        out /= cnt[:, None]
    else:
        out = np.sum([r["out"] for r in res.results], axis=0, dtype=np.float32)
    if run_kwargs:
        kernel.last_results = res
    return out


# revision 25
# speedup vs baseline: 1.1875x; 1.1875x over previous
"""Trainium2 Bass kernel for nn_CdfgReader (GNN message passing).

Strategy: the B=64 samples reference only G=8 distinct graphs, and the whole
GNN stack (input dense + 4 message-passing layers + softmax + residual) depends
only on the graph, not the sample. So each of the 8 NeuronCores computes the
full GNN for ONE graph g in [N=1024, H=256]. The per-sample masked mean is a
final [N,B]x[N,H] matmul against a host-built 0/1 mask matrix (rows zeroed for
samples of other graphs); the host sums the row-disjoint [B,H] partial
outputs and divides by the per-sample node count.

Matmul layouts avoid any on-device transpose:
  - layer: t = (A @ x)^T = matmul(lhsT=x, rhs=A^T)   (A^T fed from host)
  -        h = t^T @ W    = matmul(lhsT=t, rhs=W)
  - input: x0 = xs @ W_in = matmul(lhsT=xs^T, rhs=W_in)
  - out:   o = matmul(lhsT=maskT, rhs=x_final)

Fast path (biases zero, as in this problem): A is rescaled x20 on the host so
its entries become exactly-representable 0/1 fp8; both the A-matmul and the
W-matmul run fp8 DoubleRow; the x20 is undone by activation scale=0.05 on the
PSUM read. The mean path is also fp8; the input dense runs bf16 (fp8 xs/W_in
measurably hurts accuracy).

Schedule notes (from ntff trace analysis of the 47.8us baseline):
  - every HBM->SBUF load's completion semaphore reaches its final tick ~2.2us
    after the data lands (stores don't lag), so the DMA plan spreads the 7
    loads across the three DMA-capable queues (sync/scalar/gpsimd) ordered
    by first-consumption time: xw halves lead on sync, aT j0-3 on scalar,
    aT j4-7 on gpsimd, mT+ws trail on sync.
  - the PE clock needs ~3us of CONTINUOUS busy to reach 2.4GHz (0.65/1.2GHz
    pstates below that), and any PE idle resets the ramp; 13 dummy matmuls
    run back-to-back from the preamble until the input dense's DMA semaphore
    releases (~13us, jittery), so the ramp happens once, early.
  - W_in is stored FIRST in xw so the input dense's rhs is in the earliest
    bytes; xw is split so the first chunk releases the first 4 dense tiles.
  - engine op cost is ~200ns fixed + ~1ns/elem (vector) while scalar
    ACTIVATE scales sub-linearly; casts split vector/scalar accordingly.
  - the last layer reserves scalar for the softmax exp chain (the endgame
    bottleneck): its nch=1 chain casts go to vector, the W matmuls + exps
    carry scheduler priority, and the masked softmax mean accumulates in
    two PSUM banks so the first store overlaps the second exp half.
  - masked-mean matmuls use DoubleRow (contract 2 node tiles per pass).
  - softmax's 1/sum (~1/256, subnormal in fp8) is scaled x64 into the mask
    tile and divided back out on the host; softmax skips the max-subtraction
    (|logits| < 1 by construction).
  - NOTE for future tuning: measured exec time includes ~6.5us fixed engine
    preamble and ~8us semaphore-reset teardown, and the device thermally
    drifts +1-3us when runs are <2min apart — A/B comparisons need
    interleaved cold runs.
"""

import numpy as np
import ml_dtypes

from concourse import bacc
import concourse.mybir as mybir
import concourse.tile as tile
from concourse.bass_utils import run_bass_kernel_spmd

G, N, F, H, L, B = 8, 1024, 128, 256, 4, 64
P = 128
NT = N // P   # 8 node tiles
HT = H // P   # 2 hidden tiles
NCH = N // 512  # 2 free-dim chunks of 512 for the big matmul
NCORES = 8

F32 = mybir.dt.float32
BF16 = mybir.dt.bfloat16
F8 = mybir.dt.float8e4
PM_DR = mybir.MatmulPerfMode.DoubleRow
AX = mybir.AxisListType.X
AF = mybir.ActivationFunctionType
MUL = mybir.AluOpType.mult
MAX = mybir.AluOpType.max

# softmax 1/sum is ~1/256 — subnormal in fp8e4m3 — so the device computes
# mask*(1/sum)*MTS_SCALE and the host divides the late partials by MTS_SCALE
MTS_SCALE = 64.0

NDUM = 8  # PE-warmup matmuls: keep the clock ramping until the dense starts

_NCS = {}


def _build_nc_fast():
    """Biasless fast path: fp8 DoubleRow A- and W-matmuls, fp8 mean path."""
    nc = bacc.Bacc()
    # xw = [W_in | xs^T]: W_in leads so the dense rhs is in the first chunk
    xw = nc.dram_tensor("xw", [F, H + N], BF16, kind="ExternalInput")
    # host-pretiled aT: aT[p, j*N+n] = (A^T * 20)[j*P+p, n], fp8 0/1
    aT = nc.dram_tensor("aT", [P, NT * N], F8, kind="ExternalInput")
    # host-pretiled Ws (unscaled): ws[p, ((l*HT+c)*H)+h] = Ws[l, c*P+p, h]
    ws = nc.dram_tensor("ws", [P, L * HT * H], F8, kind="ExternalInput")
    # host-pretiled 0/1 mask (unscaled): mT[p, j*B+b] = mask[b, j*P+p]
    mT = nc.dram_tensor("mT", [P, NT * B], F8, kind="ExternalInput")
    outa = nc.dram_tensor("outa", [B, H], F32, kind="ExternalOutput")
    outb0 = nc.dram_tensor("outb0", [B, H], BF16, kind="ExternalOutput")
    outb1 = nc.dram_tensor("outb1", [B, H], BF16, kind="ExternalOutput")

    with tile.TileContext(nc) as tc:
        with (
            tc.tile_pool(name="const", bufs=1) as const,
            tc.tile_pool(name="state", bufs=2) as state,
            tc.tile_pool(name="scratch", bufs=3) as scratch,
            tc.tile_pool(name="epool", bufs=8) as epool,
            tc.tile_pool(name="mpool", bufs=8) as mpool,
            tc.tile_pool(name="ps_t", bufs=4, space="PSUM") as ps_t,
            tc.tile_pool(name="ps_h", bufs=4, space="PSUM") as ps_h,
        ):
            # ---- DMA loads: one per queue, ordered by first consumption.
            # sync: xw (split so the first dense tiles release early), mT, ws
            # scalar: aT j0-1, j2-3   gpsimd: aT j4-5, j6-7 ----
            # one big load per queue: the FIRST transfer on each queue
            # completes with fast semaphore ticks (~data rate), while later
            # transfers tick ~4x slower under notification contention — so
            # the three large loads each take a queue's fast slot
            xw_sb = const.tile([P, H + N], BF16)
            nc.sync.dma_start(xw_sb[:], xw[:])
            win_sb = xw_sb[:, 0:H]
            at_sb = const.tile([P, NT, N], F8)
            mt_sb = const.tile([P, NT, B], F8)
            ws_sb = const.tile([P, L * HT, H], F8)
            atr = aT.rearrange("p (o n) -> p o n", n=N)
            nc.scalar.dma_start(at_sb[:, 0:4, :], atr[:, 0:4, :])
            nc.gpsimd.dma_start(at_sb[:, 4:8, :], atr[:, 4:8, :])
            nc.sync.dma_start(mt_sb[:], mT.rearrange("p (o b) -> p o b", b=B))
            nc.sync.dma_start(ws_sb[:], ws.rearrange("p (c h) -> p c h", h=H))

            # ---- Exp activation-table preload (after scalar's DMA issue) ----
            warm = scratch.tile([P, 1], F32, tag="warm")
            nc.vector.memset(warm[:], 0.0)
            warm2 = scratch.tile([P, 1], F32, tag="warm2")
            nc.scalar.activation(warm2[:], warm[:], AF.Exp)

            # ---- PE warm-up: DMA-independent dummy matmuls keep the PE busy
            # so the clock ramp (0.65->1.2->2.4GHz over ~3us of continuous
            # execution) completes before the input dense ----
            dum_w = scratch.tile([P, 64], BF16, tag="dumw")
            nc.vector.memset(dum_w[:], 0.0)
            dum_r = scratch.tile([P, 512], BF16, tag="dumr")
            nc.vector.memset(dum_r[:], 0.0)
            for _ in range(NDUM):
                pdum = ps_t.tile([64, 512], F32, tag="ps_t")
                nc.tensor.matmul(pdum[:], dum_w[:], dum_r[:],
                                 start=True, stop=True)

            # ---- input dense: x0 = relu(xs @ W_in), bf16 in fp8 out ----
            x0b_sb = const.tile([P, NT, H], F8)
            for p in range(NT):
                ps = ps_h.tile([P, H], F32, tag="ps_h")
                nc.tensor.matmul(
                    ps[:], xw_sb[:, H + p * P:H + (p + 1) * P], win_sb,
                    start=True, stop=True,
                )
                # alternate relu engines so x0 j-pairs resolve fast
                if p % 2 == 0:
                    nc.vector.tensor_scalar_max(x0b_sb[:, p, :], ps[:], 0.0)
                else:
                    nc.scalar.activation(x0b_sb[:, p, :], ps[:], AF.Relu)

            x_cur = x0b_sb  # fp8 [P, NT, H]

            def cast_chain(i, nch, t_sb, ps, last=False):
                # split each PSUM->SBUF cast across vector+scalar in parallel;
                # in the last layer scalar is reserved for the softmax exps,
                # so both pieces go to vector
                base = nch * 512
                if last and nch == 1:
                    # scalar is mid-exp-chain by now; keep it free
                    nc.vector.tensor_copy(
                        t_sb[:, i, base:base + 256], ps[:, 0:256]
                    )
                    nc.vector.tensor_copy(
                        t_sb[:, i, base + 256:base + 512], ps[:, 256:512]
                    )
                elif i == 0:
                    nc.vector.tensor_copy(
                        t_sb[:, i, base:base + 256], ps[:, 0:256]
                    )
                    nc.scalar.activation(
                        t_sb[:, i, base + 256:base + 512], ps[:, 256:512],
                        AF.Copy
                    )
                else:
                    # swap engines for the i=1 chain: the W matmuls for a
                    # p-half need one piece from EACH chain, so alternating
                    # engines lets both pieces cast in parallel
                    nc.scalar.activation(
                        t_sb[:, i, base:base + 256], ps[:, 0:256], AF.Copy
                    )
                    nc.vector.tensor_copy(
                        t_sb[:, i, base + 256:base + 512], ps[:, 256:512]
                    )

            def w_relu(p, x_new, ps):
                if p % 2 == 0:
                    nc.scalar.activation(
                        x_new[:, p, :], ps[:], AF.Relu, scale=1.0 / 20.0
                    )
                else:
                    nc.vector.tensor_scalar(
                        x_new[:, p, :], ps[:], 1.0 / 20.0, 0.0, MUL, MAX
                    )

            def w_matmul(l, p):
                # p>=4 borrows the cast-drained ps_t arena so W matmuls
                # never wait on relu/exp buffer recycling in ps_h
                pool, tg = (ps_h, "ps_h") if p < 4 else (ps_t, "ps_t")
                ps = pool.tile([P, H], F32, tag=tg)
                nc.tensor.matmul(
                    ps[:],
                    t_sb[:, 0:2, p * P:(p + 1) * P],
                    ws_sb[:, l * HT:l * HT + 2, :],
                    start=True, stop=True, perf_mode=PM_DR,
                )
                return ps

            # last-layer softmax pieces
            es, mts = [], []

            def softmax_exp(l, p):
                ps = w_matmul(l, p)
                e = epool.tile([P, H], F8, tag="e")
                ssum = scratch.tile([P, 1], F32, tag="ssum")
                nc.scalar.activation(
                    e[:], ps[:], AF.Exp, scale=1.0 / 20.0, accum_out=ssum[:],
                )
                es.append(e)
                return ssum

            def softmax_norm(p, ssum):
                rinv = scratch.tile([P, 1], F32, tag="rinv")
                nc.vector.reciprocal(rinv[:], ssum[:])
                mt = mpool.tile([P, B], F8, tag="mts")
                nc.vector.tensor_scalar(
                    mt[:], mt_sb[:, p, :], rinv[:], MTS_SCALE, MUL, MUL,
                )
                mts.append(mt)

            # ---- message-passing layers ----
            for l in range(L):
                t_sb = state.tile([P, HT, N], F8, tag="t")
                if l == 0:
                    # j-outer: consume at tiles as the DMA delivers them
                    CH = [(0, 0), (1, 0), (0, 1), (1, 1)]
                    chains = {}
                    for i, nch in CH:
                        chains[(i, nch)] = ps_t.tile(
                            [P, 512], F32, tag="ps_t", name=f"pt0_{i}{nch}"
                        )
                    for j in range(0, NT, 2):
                        for i, nch in CH:
                            nc.tensor.matmul(
                                chains[(i, nch)][:],
                                x_cur[:, j:j + 2, i * P:(i + 1) * P].opt(),
                                at_sb[:, j:j + 2, nch * 512:(nch + 1) * 512].opt(),
                                start=(j == 0), stop=(j + 2 == NT),
                                perf_mode=PM_DR,
                            )
                    # masked mean, part 1: pso_a = mT^T @ x0 (DoubleRow)
                    # fills the cast gap on the PE; its store overlaps the
                    # remaining layers
                    pso_a = ps_h.tile([B, H], F32, tag="ps_h")
                    for j in range(0, NT, 2):
                        nc.tensor.matmul(
                            pso_a[:],
                            mt_sb[:, j:j + 2, :],
                            x_cur[:, j:j + 2, :],
                            start=(j == 0), stop=(j + 2 == NT),
                            perf_mode=PM_DR,
                        )
                    for i, nch in CH:
                        cast_chain(i, nch, t_sb, chains[(i, nch)])
                    oa_sb = scratch.tile([B, H], F32, tag="oa")
                    nc.vector.tensor_copy(oa_sb[:], pso_a[:])
                    nc.sync.dma_start(outa[:], oa_sb[:])
                    x_new = state.tile([P, NT, H], F8, tag="x")
                    for p in range(NT):
                        w_relu(p, x_new, w_matmul(l, p))
                    x_cur = x_new
                else:
                    # half-interleaved: A-half nch, casts, W p-half, repeat.
                    # The W matmuls' cast deps resolve while the second
                    # A-half streams, so the PE never drains.
                    last = l == L - 1
                    x_new = None
                    if not last:
                        x_new = state.tile([P, NT, H], F8, tag="x")
                    for nch in range(NCH):
                        for i in range(HT):
                            ps = ps_t.tile([P, 512], F32, tag="ps_t")
                            for j in range(0, NT, 2):
                                nc.tensor.matmul(
                                    ps[:],
                                    x_cur[:, j:j + 2, i * P:(i + 1) * P].opt(),
                                    at_sb[:, j:j + 2, nch * 512:(nch + 1) * 512].opt(),
                                    start=(j == 0), stop=(j + 2 == NT),
                                    perf_mode=PM_DR,
                                )
                            cast_chain(i, nch, t_sb, ps, last=last)
                        # (a) the last layer's W matmuls + exps get scheduler
                        # priority so they slot into the A-stream as soon as
                        # their cast deps resolve: the scalar exp chain (the
                        # endgame bottleneck) then starts mid-A-phase
                        if last:
                            ssums = []
                            with tc.high_priority():
                                for p in range(nch * 4, nch * 4 + 4):
                                    ssums.append(softmax_exp(l, p))
                            for k, p in enumerate(range(nch * 4, nch * 4 + 4)):
                                softmax_norm(p, ssums[k])
                        else:
                            for p in range(nch * 4, nch * 4 + 4):
                                w_relu(p, x_new, w_matmul(l, p))
                        if last and nch == 0:
                            # masked softmax mean, first half: accumulate
                            # p0-3 while the second A-half runs, store early
                            pso_b0 = ps_h.tile([B, H], F32, tag="ps_h")
                            for p in range(4):
                                nc.tensor.matmul(
                                    pso_b0[:], mts[p][:], es[p][:],
                                    start=(p == 0), stop=(p == 3),
                                )
                            ob0_sb = scratch.tile([B, H], BF16, tag="ob0")
                            nc.vector.tensor_copy(ob0_sb[:], pso_b0[:])
                            nc.gpsimd.dma_start(outb0[:], ob0_sb[:])
                    if not last:
                        x_cur = x_new

            # masked softmax mean, second half
            pso_b1 = ps_h.tile([B, H], F32, tag="ps_h")
            for p in range(4, NT):
                nc.tensor.matmul(
                    pso_b1[:], mts[p][:], es[p][:],
                    start=(p == 4), stop=(p == NT - 1),
                )
            ob1_sb = scratch.tile([B, H], BF16, tag="ob1")
            nc.vector.tensor_copy(ob1_sb[:], pso_b1[:])
            nc.sync.dma_start(outb1[:], ob1_sb[:])

    nc.compile()
    return nc


def _build_nc_biased():
    """General path (nonzero biases): all-f32r, bias adds on DVE."""
    F32R = mybir.dt.float32r
    nc = bacc.Bacc()
    xT = nc.dram_tensor("xT", [F, N], F32R, kind="ExternalInput")
    aT = nc.dram_tensor("aT", [N, N], F32R, kind="ExternalInput")
    win = nc.dram_tensor("win", [F, H], F32R, kind="ExternalInput")
    bin_ = nc.dram_tensor("bin", [H], F32, kind="ExternalInput")
    ws = nc.dram_tensor("ws", [L, H, H], F32R, kind="ExternalInput")
    bsd = nc.dram_tensor("bs", [L, H], F32, kind="ExternalInput")
    mT = nc.dram_tensor("mT", [N, B], F32R, kind="ExternalInput")
    out = nc.dram_tensor("out", [B, H], F32, kind="ExternalOutput")

    with tile.TileContext(nc) as tc:
        with (
            tc.tile_pool(name="const", bufs=1) as const,
            tc.tile_pool(name="state", bufs=2) as state,
            tc.tile_pool(name="scratch", bufs=3) as scratch,
            tc.tile_pool(name="ps_t", bufs=4, space="PSUM") as ps_t,
            tc.tile_pool(name="ps_h", bufs=4, space="PSUM") as ps_h,
        ):
            xt_sb = const.tile([P, N], F32R)
            nc.sync.dma_start(xt_sb[:], xT[:])
            win_sb = const.tile([P, H], F32R)
            nc.sync.dma_start(win_sb[:], win[:])
            mt_sb = const.tile([P, NT, B], F32R)
            nc.sync.dma_start(mt_sb[:], mT.rearrange("(o p) b -> p o b", p=P))
            ws_sb = const.tile([P, L * HT, H], F32R)
            nc.sync.dma_start(ws_sb[:], ws.rearrange("l (c p) h -> p (l c) h", p=P))
            bin_sb = const.tile([P, H], F32)
            nc.sync.dma_start(bin_sb[:], bin_[None, :].broadcast_to([P, H]))
            bs_sb = const.tile([P, L, H], F32)
            for l in range(L):
                nc.sync.dma_start(
                    bs_sb[:, l, :], bsd[l][None, :].broadcast_to([P, H])
                )
            at_sb = const.tile([P, NT, N], F32R)
            for j in range(NT):
                nc.sync.dma_start(at_sb[:, j, :], aT[j * P:(j + 1) * P, :])

            x0_sb = const.tile([P, NT, H], F32R)
            for p in range(NT):
                ps = ps_h.tile([P, H], F32, tag="ps_h")
                nc.tensor.matmul(
                    ps[:], xt_sb[:, p * P:(p + 1) * P], win_sb[:],
                    start=True, stop=True,
                )
                h = scratch.tile([P, H], F32, tag="hadd")
                nc.vector.tensor_add(h[:], ps[:], bin_sb[:])
                nc.scalar.activation(x0_sb[:, p, :], h[:], AF.Relu)

            x_cur = x0_sb

            for l in range(L):
                t_sb = state.tile([P, HT, N], F32R, tag="t")
                for i in range(HT):
                    for nch in range(NCH):
                        ps = ps_t.tile([P, 512], F32, tag="ps_t")
                        for j in range(NT):
                            nc.tensor.matmul(
                                ps[:],
                                x_cur[:, j, i * P:(i + 1) * P],
                                at_sb[:, j, nch * 512:(nch + 1) * 512],
                                start=(j == 0), stop=(j == NT - 1),
                            )
                        nc.any.tensor_copy(
                            t_sb[:, i, nch * 512:(nch + 1) * 512], ps[:]
                        )
                x_new = state.tile([P, NT, H], F32R, tag="x")
                for p in range(NT):
                    ps = ps_h.tile([P, H], F32, tag="ps_h")
                    for c in range(HT):
                        nc.tensor.matmul(
                            ps[:],
                            t_sb[:, c, p * P:(p + 1) * P],
                            ws_sb[:, l * HT + c, :],
                            start=(c == 0), stop=(c == HT - 1),
                        )
                    h = scratch.tile([P, H], F32, tag="hadd")
                    nc.vector.tensor_add(h[:], ps[:], bs_sb[:, l, :])
                    if l < L - 1:
                        nc.scalar.activation(x_new[:, p, :], h[:], AF.Relu)
                    else:
                        negmax = scratch.tile([P, 1], F32, tag="negmax")
                        nc.vector.reduce_max(negmax[:], h[:], axis=AX, negate=True)
                        e = scratch.tile([P, H], F32, tag="e")
                        ssum = scratch.tile([P, 1], F32, tag="ssum")
                        nc.scalar.activation(
                            e[:], h[:], AF.Exp, bias=negmax[:], accum_out=ssum[:]
                        )
                        rinv = scratch.tile([P, 1], F32, tag="rinv")
                        nc.vector.reciprocal(rinv[:], ssum[:])
                        sm = scratch.tile([P, H], F32, tag="sm")
                        nc.vector.tensor_scalar_mul(sm[:], e[:], rinv[:])
                        nc.vector.tensor_add(x_new[:, p, :], sm[:], x0_sb[:, p, :])
                x_cur = x_new

            pso = ps_h.tile([B, H], F32, tag="ps_h")
            for j in range(NT):
                nc.tensor.matmul(
                    pso[:], mt_sb[:, j, :], x_cur[:, j, :],
                    start=(j == 0), stop=(j == NT - 1),
                )
            o_sb = scratch.tile([B, H], F32, tag="o")
            nc.any.tensor_copy(o_sb[:], pso[:])
            nc.sync.dma_start(out[:], o_sb[:])

    nc.compile()
    return nc


def get_nc(variant):
    if variant not in _NCS:
        if variant == "fast8":
            _NCS[variant] = _build_nc_fast()
        else:
            _NCS[variant] = _build_nc_biased()
    return _NCS[variant]


def make_in_maps(graph, coverpoint_mask, cdfg_xs, cdfg_as, W_in, b_in, Ws, bs,
                 variant):
    graph = np.asarray(graph)
    mask = np.asarray(coverpoint_mask)
    xs = np.ascontiguousarray(np.asarray(cdfg_xs, dtype=np.float32))
    As = np.asarray(cdfg_as, dtype=np.float32)
    W_in = np.ascontiguousarray(np.asarray(W_in, dtype=np.float32))
    b_in = np.ascontiguousarray(np.asarray(b_in, dtype=np.float32))
    Ws = np.ascontiguousarray(np.asarray(Ws, dtype=np.float32))
    bs = np.ascontiguousarray(np.asarray(bs, dtype=np.float32))

    if variant == "fast8":
        # [P, L*HT*H]: ws_t[p, ((l*HT+c)*H)+h] = Ws[l, c*P+p, h]  (unscaled)
        ws_dev = np.ascontiguousarray(
            Ws.reshape(L, HT, P, H)
            .transpose(2, 0, 1, 3)
            .reshape(P, L * HT * H)
            .astype(ml_dtypes.float8_e4m3)
        )
        win_dev = W_in.astype(ml_dtypes.bfloat16)
    else:
        cnt = np.maximum(mask.sum(axis=1), 1.0).astype(np.float32)
        scaled = mask.astype(np.float32) / cnt[:, None]

    in_maps = []
    for g in range(NCORES):
        sel = graph == g
        if variant == "fast8":
            mTg = np.where(sel[:, None], mask, False).T.astype(np.float32)
            m = {
                "xw": np.ascontiguousarray(
                    np.concatenate(
                        [win_dev, xs[g].T.astype(ml_dtypes.bfloat16)],
                        axis=1,
                    )
                ),
                "ws": ws_dev,
                # [P, NT*N]: aT_t[p, j*N+n] = (A^T*20)[j*P+p, n], exact 0/1 fp8
                "aT": np.ascontiguousarray(
                    (As[g].T * 20.0)
                    .reshape(NT, P, N)
                    .transpose(1, 0, 2)
                    .reshape(P, NT * N)
                    .astype(ml_dtypes.float8_e4m3)
                ),
                # [P, NT*B]: mt_t[p, j*B+b] = mTg[j*P+p, b], exact 0/1 fp8
                "mT": np.ascontiguousarray(
                    mTg.reshape(NT, P, B)
                    .transpose(1, 0, 2)
                    .reshape(P, NT * B)
                    .astype(ml_dtypes.float8_e4m3)
                ),
            }
        else:
            mTg = np.ascontiguousarray(np.where(sel[:, None], scaled, 0.0).T)
            m = {
                "xT": np.ascontiguousarray(xs[g].T),
                "win": W_in,
                "mT": mTg.astype(np.float32),
                "aT": np.ascontiguousarray(As[g].T),
                "ws": Ws,
                "bin": b_in,
                "bs": bs,
            }
        in_maps.append(m)
    return in_maps


def kernel(graph, coverpoint_mask, cdfg_xs, cdfg_as, W_in, b_in, Ws, bs,
           **run_kwargs):
    biasless = not (np.any(np.asarray(b_in)) or np.any(np.asarray(bs)))
    variant = "fast8" if biasless else "biased"
    in_maps = make_in_maps(
        graph, coverpoint_mask, cdfg_xs, cdfg_as, W_in, b_in, Ws, bs, variant
    )
    nc = get_nc(variant)
    res = run_bass_kernel_spmd(
        nc, in_maps, core_ids=list(range(NCORES)), **run_kwargs
    )
    if variant == "fast8":
        out = np.zeros((B, H), dtype=np.float32)
        for r in res.results:
            out += r["outa"]
            out += (r["outb0"].astype(np.float32)
                    + r["outb1"].astype(np.float32)) / MTS_SCALE
        cnt = np.maximum(
            np.asarray(coverpoint_mask).sum(axis=1), 1.0
        ).astype(np.float32)
        out /= cnt[:, None]
    else:
        out = np.sum([r["out"] for r in res.results], axis=0, dtype=np.float32)
    if run_kwargs:
        kernel.last_results = res
    return out
